# revision 9
# baseline (speedup 1.0000x reference)
"""Trainium2 Bass kernel for DFlashAttention (cross+self attention, GQA, RMSNorm+RoPE).

Sharding: tensor-parallel over heads across 8 NeuronCores.
Each core owns 2 query heads + the 1 KV head they share (GQA groups=2).
q/k/v/k_ctx/v_ctx projections split column-wise, o_proj row-wise;
partial outputs are summed on the host.

Self-contained: hardcodes all shapes from the problem spec.
"""
import sys

sys.path.insert(0, "/opt/trn_rl_repo")

import numpy as np

import concourse.bacc as bacc
import concourse.mybir as mybir
import concourse.tile as tile
from concourse.bass_utils import run_bass_kernel_spmd

F32 = mybir.dt.float32
F32R = mybir.dt.float32r
AF = mybir.ActivationFunctionType
ALU = mybir.AluOpType

H, KVH, HD, HID = 16, 8, 128, 2048
S = 2048          # query tokens
L = 2048          # context tokens
T = L + S         # total keys
NCORES = 8
QH = H // NCORES  # 2 query heads per core
DLOC = QH * HD    # 256 local head dims
THETA = 10000.0
EPS = 1e-6
SCALING = HD ** -0.5

CHUNK = 512                    # token chunk
NCH = S // CHUNK               # 4 chunks
TPC = CHUNK // 128             # 4 token-tiles per chunk
NHT = HID // 128               # 16 hid tiles
NKT = T // 128                 # 32 key tiles
HIDC = HID // CHUNK            # 4 hid chunks for o_proj


def _build_nc():
    nc = bacc.Bacc("TRN2", target_bir_lowering=False, debug=False,
                   enable_asserts=False, num_devices=NCORES)

    # DRAM I/O (per-core)
    hs_t = nc.dram_tensor("hs_t", [HID, S], F32, kind="ExternalInput").ap()
    ctx_t = nc.dram_tensor("ctx_t", [HID, L], F32, kind="ExternalInput").ap()
    wq_t = nc.dram_tensor("wq_t", [HID, DLOC], F32, kind="ExternalInput").ap()
    wkv_t = nc.dram_tensor("wkv_t", [HID, 2 * HD], F32, kind="ExternalInput").ap()
    wkvc_t = nc.dram_tensor("wkvc_t", [HID, 2 * HD], F32, kind="ExternalInput").ap()
    wo_t = nc.dram_tensor("wo_t", [DLOC, HID], F32, kind="ExternalInput").ap()
    wcos_q = nc.dram_tensor("wcos_q", [S, HD], F32, kind="ExternalInput").ap()
    wsin_q = nc.dram_tensor("wsin_q", [S, HD], F32, kind="ExternalInput").ap()
    wcos_k = nc.dram_tensor("wcos_k", [S, HD], F32, kind="ExternalInput").ap()
    wsin_k = nc.dram_tensor("wsin_k", [S, HD], F32, kind="ExternalInput").ap()
    eye = nc.dram_tensor("eye", [128, 128], F32, kind="ExternalInput").ap()
    onesd = nc.dram_tensor("onesd", [128, 128], F32, kind="ExternalInput").ap()
    out_p = nc.dram_tensor("out_p", [S, HID], F32, kind="ExternalOutput").ap()

    with tile.TileContext(nc) as tc, nc.allow_low_precision(reason="fp32r matmul tiles"):
        with tc.tile_pool(name="wpool", bufs=1) as wp, \
             tc.tile_pool(name="wa", bufs=NHT) as wa, \
             tc.tile_pool(name="wb", bufs=NHT) as wb, \
             tc.tile_pool(name="state", bufs=1) as st, \
             tc.tile_pool(name="io", bufs=18) as io, \
             tc.tile_pool(name="tab", bufs=4) as tabp, \
             tc.tile_pool(name="work", bufs=3) as wk, \
             tc.tile_pool(name="ps", bufs=3, space="PSUM") as psA, \
             tc.tile_pool(name="psB", bufs=2, space="PSUM") as psB, \
             tc.tile_pool(name="psC", bufs=2, space="PSUM") as psC:

            # ---- constants / persistent weights ----
            eye_sb = wp.tile([128, 128], F32R, tag="eye")
            nc.sync.dma_start(eye_sb[:], eye.bitcast(F32R))
            ones_col = wp.tile([128, 1], F32R, tag="onescol")
            nc.sync.dma_start(ones_col[:], onesd[:, 0:1].bitcast(F32R))
            ones_row = wp.tile([1, 128], F32R, tag="onesrow")
            nc.sync.dma_start(ones_row[:], onesd[0:1, :].bitcast(F32R))

            wo_sb = wp.tile([128, QH * HID], F32R, tag="wo")
            for h in range(QH):
                nc.sync.dma_start(wo_sb[:, h * HID:(h + 1) * HID],
                                  wo_t[h * 128:(h + 1) * 128, :].bitcast(F32R))

            # persistent activation state
            KT = st.tile([128, T], F32R, tag="KT")          # K^T (d-major)
            QT0 = st.tile([128, S], F32R, tag="QT0")        # Q^T head 0
            QT1 = st.tile([128, S], F32R, tag="QT1")        # Q^T head 1
            VA = st.tile([128, T], F32R, tag="VA")          # V (token-major per key-tile)

            def proj_psum(ps_tile, src_tiles, w_sb, tt):
                """ps_tile[128tok, N] += hsT.T @ W over 16 hid tiles."""
                for i in range(NHT):
                    nc.tensor.matmul(
                        ps_tile[:],
                        src_tiles[i][:, tt * 128:(tt + 1) * 128],
                        w_sb[i][:],
                        start=(i == 0), stop=(i == NHT - 1))

            # ---------- stage B: projections ----------
            # context chunks first (K_ctx/V_ctx, no norm/rope)
            wkvc_sb = [wa.tile([128, 2 * HD], F32R, tag="wa", name=f"wkvc{i}") for i in range(NHT)]
            for i in range(NHT):
                nc.sync.dma_start(wkvc_sb[i][:],
                                  wkvc_t[i * 128:(i + 1) * 128, :].bitcast(F32R))
            for c in range(NCH):
                src = [io.tile([128, CHUNK], F32R, tag="src", name=f"src{i}") for i in range(NHT)]
                for i in range(NHT):
                    nc.sync.dma_start(
                        src[i][:],
                        ctx_t[i * 128:(i + 1) * 128,
                              c * CHUNK:(c + 1) * CHUNK].bitcast(F32R))
                for tt in range(TPC):
                    kv_ps = psA.tile([128, 2 * HD], F32, tag="psA")
                    proj_psum(kv_ps, src, wkvc_sb, tt)
                    kt = 4 * c + tt  # key-tile index (ctx occupies 0..15)
                    # K_ctx: copy then PE-transpose into KT
                    kc = wk.tile([128, 128], F32R, tag="kc")
                    nc.vector.tensor_copy(kc[:], kv_ps[:, 0:HD])
                    tp = psB.tile([128, 128], F32, tag="psB")
                    nc.tensor.transpose(tp[:].bitcast(F32R), kc[:], eye_sb[:])
                    nc.vector.tensor_copy(KT[:, kt * 128:(kt + 1) * 128], tp[:])
                    # V_ctx: straight copy (token-major)
                    nc.vector.tensor_copy(VA[:, kt * 128:(kt + 1) * 128], kv_ps[:, HD:2 * HD])

            # hidden chunks: Q (norm+rope), K (norm+rope), V
            wq_sb = [wa.tile([128, DLOC], F32R, tag="wa", name=f"wq{i}") for i in range(NHT)]
            wkv_sb = [wb.tile([128, 2 * HD], F32R, tag="wb", name=f"wkv{i}") for i in range(NHT)]
            for i in range(NHT):
                nc.sync.dma_start(wq_sb[i][:],
                                  wq_t[i * 128:(i + 1) * 128, :].bitcast(F32R))
                nc.sync.dma_start(wkv_sb[i][:],
                                  wkv_t[i * 128:(i + 1) * 128, :].bitcast(F32R))

            def nr(src_ps_slice, ctile, stile, dst_slice):
                """RMSNorm + RoPE [128tok,128d] PSUM slice -> dst (f32r sbuf)."""
                hw = HD // 2
                qn = wk.tile([128, 128], F32, tag="qn")
                nc.vector.tensor_copy(qn[:], src_ps_slice)
                sq = wk.tile([128, 128], F32, tag="sq")
                nc.vector.tensor_mul(sq[:], qn[:], qn[:])
                ssq = wk.tile([128, 1], F32, tag="ssq")
                nc.vector.tensor_reduce(ssq[:], sq[:], axis=mybir.AxisListType.X,
                                        op=ALU.add)
                ssqe = wk.tile([128, 1], F32, tag="ssqe")
                nc.vector.tensor_scalar_add(ssqe[:], ssq[:], float(HD * EPS))
                vinv = wk.tile([128, 1], F32, tag="vinv")
                nc.vector.reciprocal(vinv[:], ssqe[:])
                rstd = wk.tile([128, 1], F32, tag="rstd")
                # rstd = sqrt(HD * vinv) = 1/sqrt(mean(q^2) + eps)
                nc.scalar.activation(rstd[:], vinv[:], AF.Sqrt, scale=float(HD))
                c1 = wk.tile([128, 128], F32, tag="c1")
                nc.vector.scalar_tensor_tensor(
                    out=c1[:], in0=qn[:], scalar=rstd[:], in1=ctile[:],
                    op0=ALU.mult, op1=ALU.mult)
                c2 = wk.tile([128, 128], F32, tag="c2")
                nc.vector.scalar_tensor_tensor(
                    out=c2[:, 0:hw], in0=qn[:, hw:HD], scalar=rstd[:],
                    in1=stile[:, 0:hw], op0=ALU.mult, op1=ALU.mult)
                nc.vector.scalar_tensor_tensor(
                    out=c2[:, hw:HD], in0=qn[:, 0:hw], scalar=rstd[:],
                    in1=stile[:, hw:HD], op0=ALU.mult, op1=ALU.mult)
                rop = wk.tile([128, 128], F32R, tag="rop")
                nc.vector.tensor_add(rop[:], c1[:], c2[:])
                tp = psB.tile([128, 128], F32, tag="psB")
                nc.tensor.transpose(tp[:].bitcast(F32R), rop[:], eye_sb[:])
                nc.vector.tensor_copy(dst_slice, tp[:])

            for c in range(NCH):
                src = [io.tile([128, CHUNK], F32R, tag="src", name=f"src{i}") for i in range(NHT)]
                for i in range(NHT):
                    nc.sync.dma_start(
                        src[i][:],
                        hs_t[i * 128:(i + 1) * 128,
                             c * CHUNK:(c + 1) * CHUNK].bitcast(F32R))
                for tt in range(TPC):
                    t0 = c * CHUNK + tt * 128
                    cq = tabp.tile([128, HD], F32, tag="cq")
                    sqt = tabp.tile([128, HD], F32, tag="sqt")
                    ck = tabp.tile([128, HD], F32, tag="ck")
                    skt = tabp.tile([128, HD], F32, tag="skt")
                    nc.sync.dma_start(cq[:], wcos_q[t0:t0 + 128, :])
                    nc.sync.dma_start(sqt[:], wsin_q[t0:t0 + 128, :])
                    nc.sync.dma_start(ck[:], wcos_k[t0:t0 + 128, :])
                    nc.sync.dma_start(skt[:], wsin_k[t0:t0 + 128, :])

                    q_ps = psA.tile([128, DLOC], F32, tag="psA")
                    proj_psum(q_ps, src, wq_sb, tt)
                    nr(q_ps[:, 0:HD], cq, sqt, QT0[:, t0:t0 + 128])
                    nr(q_ps[:, HD:2 * HD], cq, sqt, QT1[:, t0:t0 + 128])

                    kv_ps = psA.tile([128, 2 * HD], F32, tag="psA")
                    proj_psum(kv_ps, src, wkv_sb, tt)
                    kt = 16 + 4 * c + tt  # self keys at tiles 16..31
                    nr(kv_ps[:, 0:HD], ck, skt, KT[:, kt * 128:(kt + 1) * 128])
                    nc.vector.tensor_copy(VA[:, kt * 128:(kt + 1) * 128], kv_ps[:, HD:2 * HD])

            # ---------- stage C: attention + o_proj ----------
            for qc in range(NCH):
                q0 = qc * CHUNK
                attT = []   # [d=128, 512] per head, post 1/l
                for h in range(QH):
                    QTh = QT0 if h == 0 else QT1
                    att_ps = psC.tile([128, CHUNK], F32, tag="psC")
                    l_ps = psC.tile([1, CHUNK], F32, tag="psC")
                    for kt in range(NKT):
                        sT = psA.tile([128, CHUNK], F32, tag="psA")
                        nc.tensor.matmul(
                            sT[:], KT[:, kt * 128:(kt + 1) * 128],
                            QTh[:, q0:q0 + CHUNK], start=True, stop=True)
                        pT = wk.tile([128, CHUNK], F32R, tag="pT")
                        nc.scalar.activation(pT[:], sT[:], AF.Exp, scale=SCALING)
                        nc.tensor.matmul(
                            att_ps[:], VA[:, kt * 128:(kt + 1) * 128], pT[:],
                            start=(kt == 0), stop=(kt == NKT - 1))
                        nc.tensor.matmul(
                            l_ps[:], ones_col[:], pT[:],
                            start=(kt == 0), stop=(kt == NKT - 1))
                    rl_row = wk.tile([1, CHUNK], F32R, tag="rlrow")
                    nc.vector.reciprocal(rl_row[:], l_ps[:])
                    rlb_ps = psB.tile([128, CHUNK], F32, tag="psB")
                    nc.tensor.matmul(rlb_ps[:], ones_row[:], rl_row[:],
                                     start=True, stop=True)
                    rl_b = wk.tile([128, CHUNK], F32, tag="rlb")
                    nc.scalar.copy(rl_b[:], rlb_ps[:])
                    aT = wk.tile([128, CHUNK], F32R, tag="attT")
                    nc.vector.tensor_mul(aT[:], att_ps[:], rl_b[:])
                    attT.append(aT)
                for j in range(TPC):
                    for hc in range(HIDC):
                        o_ps = psA.tile([128, CHUNK], F32, tag="psA")
                        for h in range(QH):
                            nc.tensor.matmul(
                                o_ps[:],
                                attT[h][:, j * 128:(j + 1) * 128],
                                wo_sb[:, h * HID + hc * CHUNK:
                                      h * HID + (hc + 1) * CHUNK],
                                start=(h == 0), stop=(h == QH - 1))
                        ot = wk.tile([128, CHUNK], F32, tag="ot")
                        nc.vector.tensor_copy(ot[:], o_ps[:])
                        nc.sync.dma_start(
                            out_p[q0 + j * 128:q0 + (j + 1) * 128,
                                  hc * CHUNK:(hc + 1) * CHUNK], ot[:])

    nc.compile()
    return nc


_NC_CACHE = {}


def _get_nc():
    if "nc" not in _NC_CACHE:
        _NC_CACHE["nc"] = _build_nc()
    return _NC_CACHE["nc"]


def _host_prep(hidden_states, context, position_ids, Wq, Wk, Wv, Wo,
               Wk_ctx, Wv_ctx, q_norm_w, k_norm_w):
    f32 = np.float32
    hs_t = np.ascontiguousarray(hidden_states[0].T, dtype=f32)
    ctx_t = np.ascontiguousarray(context[0].T, dtype=f32)

    # RoPE tables with norm weights + rotate-half sign folded in
    pos = np.asarray(position_ids[0], dtype=np.float64)
    inv_freq = 1.0 / (THETA ** (np.arange(0, HD, 2, dtype=np.float64) / HD))
    freqs = pos[:, None] * inv_freq[None, :]          # (S, 64)
    emb = np.concatenate([freqs, freqs], axis=1)      # (S, 128)
    cos_t = np.cos(emb).astype(f32)
    sin_t = np.sin(emb).astype(f32)
    hw = HD // 2

    def tables(w):
        w = np.asarray(w, dtype=f32)
        wcos = cos_t * w[None, :]
        wsin = np.empty_like(sin_t)
        wsin[:, :hw] = -sin_t[:, :hw] * w[None, hw:]
        wsin[:, hw:] = sin_t[:, hw:] * w[None, :hw]
        return np.ascontiguousarray(wcos), np.ascontiguousarray(wsin)

    wcos_q, wsin_q = tables(q_norm_w)
    wcos_k, wsin_k = tables(k_norm_w)

    eye = np.eye(128, dtype=f32)
    onesd = np.ones((128, 128), dtype=f32)

    in_maps = []
    for c in range(NCORES):
        qs = slice(c * DLOC, (c + 1) * DLOC)
        ks = slice(c * HD, (c + 1) * HD)
        wq_t = np.ascontiguousarray(Wq[qs, :].T, dtype=f32)
        wkv_t = np.ascontiguousarray(
            np.concatenate([Wk[ks, :], Wv[ks, :]], axis=0).T, dtype=f32)
        wkvc_t = np.ascontiguousarray(
            np.concatenate([Wk_ctx[ks, :], Wv_ctx[ks, :]], axis=0).T, dtype=f32)
        wo_t = np.ascontiguousarray(Wo[:, qs].T, dtype=f32)
        in_maps.append({
            "hs_t": hs_t, "ctx_t": ctx_t,
            "wq_t": wq_t, "wkv_t": wkv_t, "wkvc_t": wkvc_t, "wo_t": wo_t,
            "wcos_q": wcos_q, "wsin_q": wsin_q,
            "wcos_k": wcos_k, "wsin_k": wsin_k,
            "eye": eye, "onesd": onesd,
        })
    return in_maps


def kernel(**inputs):
    in_maps = _host_prep(**inputs)
    nc = _get_nc()
    res = run_bass_kernel_spmd(nc, in_maps, core_ids=list(range(NCORES)))
    out = np.zeros((S, HID), dtype=np.float64)
    for c in range(NCORES):
        out += res.results[c]["out_p"].astype(np.float64)
    return out[None, :, :].astype(np.float32)
